# revision 6
# baseline (speedup 1.0000x reference)
"""Trainium2 Bass kernel: gated-cross-attention transformer decoder layer.

Sharding: data-parallel over batch B=8 -> one batch element per NeuronCore,
weights replicated, no collectives.

Per-core layout strategy:
  - Activations that feed matmul contractions are kept/produced transposed
    ([feature, token] with feature on partitions); LayerNorm/softmax outputs
    are kept natural ([token, feature]) so row reductions run on the free dim.
  - Attention probabilities are computed transposed (scores^T[s, t]); the
    softmax denominator comes for free by augmenting V with a ones column,
    and the 1/rowsum renormalization is applied via a PE outer-product
    broadcast.
  - All weights are pre-transposed on the host, so no on-chip weight
    transposes are needed. Only x1/x2 (LN outputs) are PE-transposed.
  - Matmuls run as float32r (full-rate fp32 streaming).

A (512, C) matrix is packed host-side as (128, 4, C): partition p, tile i
holds row 128*i + p.
"""

from contextlib import ExitStack

import numpy as np

import concourse.bass as bass
import concourse.mybir as mybir
import concourse.tile as tile
from concourse import bacc
from concourse.bass_utils import run_bass_kernel_spmd
from concourse.masks import make_identity

B, T, S, D, H = 8, 512, 512, 512, 8
DH = D // H          # 64
F = 4 * D            # 2048
P = 128
NT, ND, NF = T // P, D // P, F // P   # 4, 4, 16
EPS = 1e-5
FP32 = mybir.dt.float32
F32R = mybir.dt.float32r
AF = mybir.ActivationFunctionType
OP = mybir.AluOpType




# ---------------------------------------------------------------------------
# device program
# ---------------------------------------------------------------------------

def _emit(nc, dbg=False):
    din = {}

    def dram_in(name, shape, dt=FP32):
        din[name] = nc.dram_tensor(name, list(shape), dt, kind="ExternalInput")
        return din[name]

    # per-core activations
    dram_in("tgt_n", (P, NT, D))
    dram_in("tgt_t", (P, ND, T), F32R)
    dram_in("mem_t", (P, ND, S), F32R)
    # shared
    dram_in("mask_t", (P, NT, T))       # tgt_mask.T packed: [s, t]
    dram_in("gate_t", (P, ND, T))       # gate.T packed: [d, t]
    # self-attn weights (pre-transposed to [d_in, d_out]; q already scaled)
    for w in ("wq_t", "wk_t", "wv_t", "wo_t", "cwq_t", "cwv_t", "cwo_t"):
        dram_in(w, (P, ND, D), F32R)
    dram_in("cwk_n", (P, ND, D), F32R)  # cross W_k kept natural [(h,e), d]
    dram_in("w1_t", (P, ND, F), F32R)   # ff1_w.T  [d, f]
    dram_in("w2_t", (P, NF, D), F32R)   # ff2_w.T  [f, d]
    dram_in("ones_r", (1, DH), F32R)    # fp32r ones (memset cannot write f32r)
    dram_in("vones", (P, NT, H, 2), F32R)
    # per-partition bias columns
    dram_in("bq", (P, ND))
    dram_in("bk", (P, ND))
    dram_in("cbq", (P, ND))
    dram_in("b1", (P, NF))
    # free-dim (broadcast) bias rows, pre-tiled to 128 partitions
    for b in ("bv_b", "bo_b", "cbv_b", "cbo_b", "b2_b",
              "g1_b", "b1n_b", "g2_b", "b2n_b", "g3_b", "b3n_b"):
        dram_in(b, (P, D))

    out_d = nc.dram_tensor("out", [P, NT, D], FP32, kind="ExternalOutput")
    dbg_outs = {}
    if dbg:
        for nm, shp in [("d_qT", (P, ND, T)), ("d_kT", (P, ND, T)),
                        ("d_vA", (P, NT, H, DH + 2)), ("d_oT", (P, ND, T)),
                        ("d_exp0", (P, NT, T)), ("d_x1", (P, NT, D)),
                        ("d_x1t", (P, ND, T)), ("d_cqT", (P, ND, T)),
                        ("d_gT0", (P, ND, T)), ("d_cexp0", (P, NT, T)),
                        ("d_coT", (P, ND, T)), ("d_x2", (P, NT, D)),
                        ("d_hT", (P, NF, T))]:
            dbg_outs[nm] = nc.dram_tensor(nm, list(shp), FP32,
                                          kind="ExternalOutput")

    with tile.TileContext(nc) as tc, ExitStack() as ctx, \
            nc.allow_low_precision(reason="float32r matmul operand rounding"):
        # ---- PSUM pools (8 banks total) ----
        pp_mm = ctx.enter_context(tc.tile_pool(name="pp_mm", bufs=2, space="PSUM"))
        pp_sc = ctx.enter_context(tc.tile_pool(name="pp_sc", bufs=2, space="PSUM"))
        pp_o = ctx.enter_context(tc.tile_pool(name="pp_o", bufs=1, space="PSUM"))
        pp_bc = ctx.enter_context(tc.tile_pool(name="pp_bc", bufs=1, space="PSUM"))
        pp_tr = ctx.enter_context(tc.tile_pool(name="pp_tr", bufs=2, space="PSUM"))

        sm = ctx.enter_context(tc.tile_pool(name="sm", bufs=4))
        const = ctx.enter_context(tc.tile_pool(name="const", bufs=1))

        ident = const.tile([P, P], FP32)
        make_identity(nc, ident)
        ones64 = const.tile([1, DH], F32R)
        nc.sync.dma_start(ones64, din["ones_r"][:])
        eps_t = const.tile([P, 1], FP32)
        nc.vector.memset(eps_t, EPS)

        def dump(nm, tile_):
            if dbg:
                src_ap = tile_
                if src_ap.dtype != FP32:
                    src_ap = src_ap.bitcast(FP32)
                nc.sync.dma_start(dbg_outs[nm][:], src_ap)

        def load(pool, name, tag=None):
            t = pool.tile(list(din[name].shape), din[name].dtype,
                          name=name + "_sb", tag=tag or name)
            nc.sync.dma_start(t, din[name][:])
            return t

        def proj_T(dst, w_sb, x_t, b_col=None):
            """dst[:, j, :] (transposed [d_out, t]) = W^T.T @ x^T  (+ bias)."""
            for j in range(ND):
                ps = pp_mm.tile([P, T], FP32, name="mmps", tag="mm")
                for k in range(ND):
                    nc.tensor.matmul(ps, (w_sb[:, k, j * P:(j + 1) * P]),
                                     (x_t[:, k, :]),
                                     start=(k == 0), stop=(k == ND - 1))
                if b_col is not None:
                    nc.vector.tensor_scalar_add(dst[:, j, :], ps,
                                                b_col[:, j:j + 1])
                else:
                    nc.vector.tensor_copy(dst[:, j, :], ps)

        def value_aug(dst, a_t, w_sb, b_bcast):
            """dst = ones-augmented V: dst[:, i, h, 0:DH] = (A @ W^T + b) heads.

            a_t: [P, ND, S] transposed source (contraction on partitions);
            dst: [P, NT, H, DH+1].
            """
            nc.sync.dma_start(dst[:, :, :, DH:DH + 2], din["vones"][:])
            for i in range(NT):
                ps = pp_mm.tile([P, D], FP32, name="mmps", tag="mm")
                for k in range(ND):
                    nc.tensor.matmul(ps, (a_t[:, k, i * P:(i + 1) * P]),
                                     (w_sb[:, k, :]),
                                     start=(k == 0), stop=(k == ND - 1))
                nc.vector.tensor_add(
                    dst[:, i, :, 0:DH],
                    ps.rearrange("p (h e) -> p h e", h=H),
                    b_bcast.rearrange("p (h e) -> p h e", h=H))

        def attention(qT, kT_or_scores, vA, oT, mask=None, dbg_exp=None):
            """Shared attention core.

            self-attn: kT_or_scores = kT (scores^T = kT_h^T-slice @ qT_h),
            cross-attn: kT_or_scores = callable(h, si, psum) emitting the
            score matmuls for head h / s-tile si.
            """
            for h in range(H):
                hp, ht = (h % 2) * DH, h // 2
                exp_t = phs_cur["pool"].tile([P, NT, T], F32R, name="expT",
                                             tag="expT", bufs=2)
                for si in range(NT):
                    ps = pp_sc.tile([P, T], FP32, name="scps", tag="sc")
                    if callable(kT_or_scores):
                        kT_or_scores(h, si, ps)
                    else:
                        kT = kT_or_scores
                        nc.tensor.matmul(
                            ps,
                            (kT[hp:hp + DH, ht, si * P:(si + 1) * P]),
                            (qT[hp:hp + DH, ht, :]),
                            start=True, stop=True)
                    if mask is not None:
                        nc.vector.scalar_tensor_tensor(
                            out=exp_t[:, si, :], in0=ps, scalar=1.0,
                            in1=mask[:, si, :], op0=OP.mult, op1=OP.add)
                        nc.scalar.activation(exp_t[:, si, :], exp_t[:, si, :],
                                             AF.Exp)
                    else:
                        nc.scalar.activation(exp_t[:, si, :], ps, AF.Exp)
                if h == 0 and dbg_exp:
                    dump(dbg_exp, exp_t)
                po = pp_o.tile([DH + 2, T], FP32, name="ops", tag="po")
                for si in range(NT):
                    nc.tensor.matmul(po, (vA[:, si, h, :]),
                                     (exp_t[:, si, :]),
                                     start=(si == 0), stop=(si == NT - 1))
                rec = sm.tile([1, T], F32R, name="rec", tag="rec", bufs=2)
                nc.vector.reciprocal(rec, po[DH:DH + 1, :])
                pb = pp_bc.tile([DH, T], FP32, name="bcps", tag="pb")
                nc.tensor.matmul(pb, (ones64), (rec), start=True, stop=True)
                pb_sb = phs_cur["pool"].tile([DH, T], FP32, name="pb_sb",
                                             tag="pb_sb", bufs=2)
                nc.scalar.copy(pb_sb, pb)
                nc.vector.tensor_mul(oT[hp:hp + DH, ht, :], po[0:DH, :], pb_sb)

        def out_proj_residual(oT, w_sb, b_bcast, resid, dst):
            """dst[:, ti, :] = resid + (o @ W_o^T + b) (natural layout)."""
            for ti in range(NT):
                ps = pp_mm.tile([P, D], FP32, name="mmps", tag="mm")
                for k in range(ND):
                    nc.tensor.matmul(ps, (oT[:, k, ti * P:(ti + 1) * P]),
                                     (w_sb[:, k, :]),
                                     start=(k == 0), stop=(k == ND - 1))
                nc.vector.scalar_tensor_tensor(
                    out=dst[:, ti, :], in0=ps, scalar=1.0,
                    in1=b_bcast, op0=OP.mult, op1=OP.add)
                nc.vector.tensor_add(dst[:, ti, :], dst[:, ti, :],
                                     resid[:, ti, :])

        def layer_norm(x_sb, g_b, bb_b):
            """In-place LN over the free dim of each [P, 512] tile."""
            for ti in range(NT):
                st = sm.tile([P, 6], FP32, name="st", tag="st", bufs=4)
                nc.vector.bn_stats(st, x_sb[:, ti, :])
                mv = sm.tile([P, 2], FP32, name="mv", tag="mv", bufs=4)
                nc.vector.bn_aggr(mv, st)
                sd = sm.tile([P, 1], FP32, name="sd", tag="sd", bufs=4)
                nc.scalar.activation(sd, mv[:, 1:2], AF.Sqrt, bias=eps_t)
                nc.vector.reciprocal(sd, sd)
                nc.vector.tensor_scalar(
                    out=x_sb[:, ti, :], in0=x_sb[:, ti, :],
                    scalar1=mv[:, 0:1], scalar2=sd,
                    op0=OP.subtract, op1=OP.mult)
                nc.gpsimd.tensor_mul(x_sb[:, ti, :], x_sb[:, ti, :], g_b)
                nc.gpsimd.tensor_add(x_sb[:, ti, :], x_sb[:, ti, :], bb_b)

        def transpose_full(dst, src):
            """dst = src^T for packed (512,512) tiles."""
            for i in range(NT):
                for j in range(ND):
                    pt = pp_tr.tile([P, P], FP32, name="trps", tag="pt")
                    nc.tensor.transpose(pt, src[:, i, j * P:(j + 1) * P], ident)
                    nc.vector.tensor_copy(dst[:, j, i * P:(i + 1) * P], pt)

        phs_cur = {}

        with tc.tile_pool(name="wattn", bufs=1) as wattn, \
                tc.tile_pool(name="mid1", bufs=1) as mid1:
            # attention weights up-front (DMA overlaps early compute)
            wq = load(wattn, "wq_t")
            wk = load(wattn, "wk_t")
            wv = load(wattn, "wv_t")
            wo = load(wattn, "wo_t")
            cwq = load(wattn, "cwq_t")
            cwk = load(wattn, "cwk_n")
            cwv = load(wattn, "cwv_t")
            cwo = load(wattn, "cwo_t")
            bq = load(wattn, "bq")
            bk = load(wattn, "bk")
            cbq = load(wattn, "cbq")

            x1 = mid1.tile([P, NT, D], FP32, name="x1")
            x1t = mid1.tile([P, ND, T], F32R, name="x1t")

            # ================= self attention =================
            with tc.tile_pool(name="ph_s", bufs=1) as phs:
                phs_cur["pool"] = phs
                tgt_n = load(phs, "tgt_n")
                tgt_t = load(phs, "tgt_t")
                mask_t = load(phs, "mask_t")
                bv_b = load(phs, "bv_b")
                bo_b = load(phs, "bo_b")
                g1_b = load(phs, "g1_b")
                b1n_b = load(phs, "b1n_b")

                qT = phs.tile([P, ND, T], F32R, name="qT")
                kT = phs.tile([P, ND, T], F32R, name="kT")
                vA = phs.tile([P, NT, H, DH + 2], F32R, name="vA")
                oT = phs.tile([P, ND, T], F32R, name="oT")

                proj_T(qT, wq, tgt_t, bq)
                proj_T(kT, wk, tgt_t, bk)
                value_aug(vA, tgt_t, wv, bv_b)
                attention(qT, kT, vA, oT, mask=mask_t, dbg_exp="d_exp0")
                dump("d_qT", qT)
                dump("d_kT", kT)
                dump("d_vA", vA)
                dump("d_oT", oT)
                out_proj_residual(oT, wo, bo_b, tgt_n, x1)
                layer_norm(x1, g1_b, b1n_b)
                transpose_full(x1t, x1)
                dump("d_x1", x1)
                dump("d_x1t", x1t)

            # ================= gated cross attention =================
            with tc.tile_pool(name="mid2", bufs=1) as mid2:
                x2 = mid2.tile([P, NT, D], FP32, name="x2")
                x2t = mid2.tile([P, ND, T], F32R, name="x2t")

                with tc.tile_pool(name="ph_c", bufs=1) as phc:
                    phs_cur["pool"] = phc
                    mem_t = load(phc, "mem_t")
                    gate_t = load(phc, "gate_t")
                    cbv_b = load(phc, "cbv_b")
                    cbo_b = load(phc, "cbo_b")
                    g2_b = load(phc, "g2_b")
                    b2n_b = load(phc, "b2n_b")

                    cqT = phc.tile([P, ND, T], F32R, name="cqT")
                    cvA = phc.tile([P, NT, H, DH + 2], F32R, name="cvA")
                    coT = phc.tile([P, ND, T], F32R, name="coT")

                    proj_T(cqT, cwq, x1t, cbq)
                    value_aug(cvA, mem_t, cwv, cbv_b)

                    # per-head gated keys: gT_h[d, t] = (wk_h^T @ cq_h^T) * gate^T
                    g_tiles = {}

                    def cross_scores(h, si, ps):
                        hp, ht = (h % 2) * DH, h // 2
                        if h not in g_tiles:
                            gT = phc.tile([P, ND, T], F32R, name="gT",
                                          tag="gT", bufs=2)
                            for dj in range(ND):
                                qw = pp_mm.tile([P, T], FP32, name="mmps",
                                                tag="mm")
                                nc.tensor.matmul(
                                    qw,
                                    (cwk[hp:hp + DH, ht, dj * P:(dj + 1) * P]),
                                    (cqT[hp:hp + DH, ht, :]),
                                    start=True, stop=True)
                                nc.vector.tensor_mul(gT[:, dj, :], qw,
                                                     gate_t[:, dj, :])
                            g_tiles.clear()
                            g_tiles[h] = gT
                            if h == 0:
                                dump("d_gT0", gT)
                        gT = g_tiles[h]
                        for k in range(ND):
                            nc.tensor.matmul(
                                ps, (mem_t[:, k, si * P:(si + 1) * P]),
                                (gT[:, k, :]),
                                start=(k == 0), stop=(k == ND - 1))

                    attention(cqT, cross_scores, cvA, coT, mask=None, dbg_exp="d_cexp0")
                    dump("d_cqT", cqT)
                    dump("d_coT", coT)
                    out_proj_residual(coT, cwo, cbo_b, x1, x2)
                    layer_norm(x2, g2_b, b2n_b)
                    transpose_full(x2t, x2)
                    dump("d_x2", x2)

                # ================= FFN =================
                with tc.tile_pool(name="ph_f", bufs=1) as phf:
                    w2 = load(phf, "w2_t")
                    b1 = load(phf, "b1")
                    b2_b = load(phf, "b2_b")
                    g3_b = load(phf, "g3_b")
                    b3n_b = load(phf, "b3n_b")

                    hT = phf.tile([P, NF, T], F32R, name="hT")
                    x3 = phf.tile([P, NT, D], FP32, name="x3")

                    for fj in range(NF):
                        w1c = phf.tile([P, ND, P], F32R, name="w1c",
                                       tag="w1c", bufs=4)
                        nc.sync.dma_start(
                            w1c, din["w1_t"][:, :, fj * P:(fj + 1) * P])
                        ps = pp_mm.tile([P, T], FP32, name="mmps", tag="mm")
                        for k in range(ND):
                            nc.tensor.matmul(ps, (w1c[:, k, :]),
                                             (x2t[:, k, :]),
                                             start=(k == 0), stop=(k == ND - 1))
                        nc.scalar.activation(hT[:, fj, :], ps, AF.Relu,
                                             bias=b1[:, fj:fj + 1])

                    dump("d_hT", hT)
                    for ti in range(NT):
                        ps = pp_mm.tile([P, D], FP32, name="mmps", tag="mm")
                        for k in range(NF):
                            nc.tensor.matmul(
                                ps, (hT[:, k, ti * P:(ti + 1) * P]),
                                (w2[:, k, :]),
                                start=(k == 0), stop=(k == NF - 1))
                        nc.vector.scalar_tensor_tensor(
                            out=x3[:, ti, :], in0=ps, scalar=1.0,
                            in1=b2_b, op0=OP.mult, op1=OP.add)
                        nc.vector.tensor_add(x3[:, ti, :], x3[:, ti, :],
                                             x2[:, ti, :])
                    layer_norm(x3, g3_b, b3n_b)
                    for ti in range(NT):
                        nc.sync.dma_start(out_d[:, ti, :], x3[:, ti, :])

    return nc


# ---------------------------------------------------------------------------
# host side
# ---------------------------------------------------------------------------

def _pack(m):
    """(R, C) -> (128, R//128, C): partition-major packing."""
    m = np.ascontiguousarray(m, dtype=np.float32)
    r, c = m.shape
    return np.ascontiguousarray(m.reshape(r // P, P, c).transpose(1, 0, 2))


def _col(v):
    """(N,) -> (128, N//128) per-partition bias columns."""
    v = np.asarray(v, dtype=np.float32)
    return np.ascontiguousarray(v.reshape(-1, P).T)


def _bcast(v):
    v = np.asarray(v, dtype=np.float32)
    return np.ascontiguousarray(np.broadcast_to(v, (P, v.size)))


_CACHE = {}


def _get_nc(dbg=False):
    key = ("nc", dbg)
    if key not in _CACHE:
        nc = bacc.Bacc("TRN2", target_bir_lowering=False, debug=False,
                       enable_asserts=False, num_devices=B)
        _emit(nc, dbg=dbg)
        nc.compile()
        _CACHE[key] = nc
    return _CACHE[key]


def _shared_inputs(inputs):
    scale = 1.0 / np.sqrt(np.float32(DH))
    sa_w = np.asarray(inputs["sa_in_w"], np.float32)
    sa_b = np.asarray(inputs["sa_in_b"], np.float32)
    ca_w = np.asarray(inputs["ca_in_w"], np.float32)
    ca_b = np.asarray(inputs["ca_in_b"], np.float32)
    sh = {
        "mask_t": _pack(np.asarray(inputs["tgt_mask"], np.float32).T),
        "gate_t": _pack(np.asarray(inputs["gate"], np.float32).T),
        "wq_t": _pack(sa_w[0:D].T * scale),
        "wk_t": _pack(sa_w[D:2 * D].T),
        "wv_t": _pack(sa_w[2 * D:3 * D].T),
        "wo_t": _pack(np.asarray(inputs["sa_out_w"], np.float32).T),
        "cwq_t": _pack(ca_w[0:D].T * scale),
        "cwk_n": _pack(ca_w[D:2 * D]),
        "cwv_t": _pack(ca_w[2 * D:3 * D].T),
        "cwo_t": _pack(np.asarray(inputs["ca_out_w"], np.float32).T),
        "w1_t": _pack(np.asarray(inputs["ff1_w"], np.float32).T),
        "w2_t": _pack(np.asarray(inputs["ff2_w"], np.float32).T),
        "bq": _col(sa_b[0:D] * scale),
        "bk": _col(sa_b[D:2 * D]),
        "cbq": _col(ca_b[0:D] * scale),
        "b1": _col(np.asarray(inputs["ff1_b"], np.float32)),
        "bv_b": _bcast(sa_b[2 * D:3 * D]),
        "bo_b": _bcast(np.asarray(inputs["sa_out_b"], np.float32)),
        "cbv_b": _bcast(ca_b[2 * D:3 * D]),
        "cbo_b": _bcast(np.asarray(inputs["ca_out_b"], np.float32)),
        "b2_b": _bcast(np.asarray(inputs["ff2_b"], np.float32)),
        "g1_b": _bcast(np.asarray(inputs["ln1_g"], np.float32)),
        "b1n_b": _bcast(np.asarray(inputs["ln1_b"], np.float32)),
        "g2_b": _bcast(np.asarray(inputs["ln2_g"], np.float32)),
        "b2n_b": _bcast(np.asarray(inputs["ln2_b"], np.float32)),
        "g3_b": _bcast(np.asarray(inputs["ln3_g"], np.float32)),
        "b3n_b": _bcast(np.asarray(inputs["ln3_b"], np.float32)),
        "ones_r": np.ones((1, DH), np.float32),
        "vones": np.ones((P, NT, H, 2), np.float32),
    }
    return sh


def _run(inputs, trace=False, dbg=False, cores=None):
    nc = _get_nc(dbg=dbg)
    tgt = np.asarray(inputs["tgt"], np.float32)
    memory = np.asarray(inputs["memory"], np.float32)
    sh = _shared_inputs(inputs)
    core_list = list(range(B)) if cores is None else cores
    in_maps = []
    for b in core_list:
        m = dict(sh)
        m["tgt_n"] = _pack(tgt[b])
        m["tgt_t"] = _pack(tgt[b].T)
        m["mem_t"] = _pack(memory[b].T)
        in_maps.append(m)
    res = run_bass_kernel_spmd(nc, in_maps, core_list, trace=trace)
    out = np.stack([
        res.results[i]["out"].transpose(1, 0, 2).reshape(T, D)
        for i in range(len(core_list))
    ])
    return out.astype(np.float32), res


def kernel(**inputs):
    return _run(inputs, trace=False)[0]


# revision 14
# speedup vs baseline: 632.5153x; 632.5153x over previous
"""Trainium2 Bass kernel: gated-cross-attention transformer decoder layer.

Sharding: data-parallel over batch B=8 -> one batch element per NeuronCore,
weights replicated, no collectives.

Per-core layout strategy:
  - Activations that feed matmul contractions are kept/produced transposed
    ([feature, token] with feature on partitions); LayerNorm/softmax outputs
    are kept natural ([token, feature]) so row reductions run on the free dim.
  - Attention probabilities are computed transposed (scores^T[s, t]); the
    softmax denominator comes for free by augmenting V with 64 ones columns,
    which lands the rowsum replicated across PSUM partitions 64-127, so the
    renormalization is a plain reciprocal + elementwise multiply.
  - All weights are pre-transposed on the host; LayerNorm affine transforms
    feeding matmuls are folded into the downstream weights host-side, so the
    transposed path uses the pre-affine normalized activations.
  - Matmuls run as float32r (full-rate fp32 streaming, ~12-bit mantissa).

A (512, C) matrix is packed host-side as (128, 4, C): partition p, tile i
holds row 128*i + p.
"""

from contextlib import ExitStack

import numpy as np

import concourse.bass as bass
import concourse.mybir as mybir
import concourse.tile as tile
from concourse import bacc
from concourse.bass_utils import run_bass_kernel_spmd
from concourse.masks import make_identity

B, T, S, D, H = 8, 512, 512, 512, 8
DH = D // H          # 64
F = 4 * D            # 2048
P = 128
NT, ND, NF = T // P, D // P, F // P   # 4, 4, 16
EPS = 1e-5
FP32 = mybir.dt.float32
F32R = mybir.dt.float32r
AF = mybir.ActivationFunctionType
OP = mybir.AluOpType


# ---------------------------------------------------------------------------
# device program
# ---------------------------------------------------------------------------

def _emit(nc, dbg=False, iters=1):
    din = {}

    def dram_in(name, shape, dt=FP32):
        din[name] = nc.dram_tensor(name, list(shape), dt, kind="ExternalInput")
        return din[name]

    # per-core activations
    dram_in("tgt_n", (P, NT, D))
    dram_in("tgt_t", (P, ND, T), F32R)
    dram_in("mem_t", (P, ND, S), F32R)
    # shared
    dram_in("mask_t", (P, NT, T))       # tgt_mask.T packed: [s, t]
    dram_in("gate_t", (P, ND, T))       # gate.T packed: [d, t]
    # weights (pre-transposed to [d_in, d_out]; q scaled; LN affines folded)
    for w in ("wq_t", "wk_t", "wv_t", "wo_t", "cwq_t", "cwv_t", "cwo_t"):
        dram_in(w, (P, ND, D), F32R)
    dram_in("cwk_n", (P, ND, D), F32R)  # cross W_k kept natural [(h,e), d]
    dram_in("w1_t", (P, ND, F), F32R)   # (ff1_w * g2).T  [d, f]
    dram_in("w2_t", (P, NF, D), F32R)   # ff2_w.T  [f, d]
    dram_in("vones", (P, NT, H, DH), F32R)
    # per-partition bias columns
    dram_in("bq", (P, ND))
    dram_in("bk", (P, ND))
    dram_in("cbq", (P, ND))
    dram_in("b1", (P, NF))
    # free-dim (broadcast) bias rows, pre-tiled to 128 partitions
    for b in ("bv_b", "bo_b", "cbv_b",
              "g1_b", "rb1_b", "g2_b", "rb2_b", "g3_b", "b3n_b"):
        dram_in(b, (P, D))

    out_d = nc.dram_tensor("out", [P, NT, D], FP32, kind="ExternalOutput")
    dbg_outs = {}
    if dbg:
        for nm, shp in [("d_qT", (P, ND, T)), ("d_kT", (P, ND, T)),
                        ("d_vA", (P, NT, H, 2 * DH)), ("d_oT", (P, ND, T)),
                        ("d_exp0", (P, NT, T)), ("d_x1", (P, NT, D)),
                        ("d_x1t", (P, ND, T)), ("d_cqT", (P, ND, T)),
                        ("d_gT0", (P, ND, T)), ("d_cexp0", (P, NT, T)),
                        ("d_coT", (P, ND, T)), ("d_x2", (P, NT, D)),
                        ("d_hT", (P, NF, T))]:
            dbg_outs[nm] = nc.dram_tensor(nm, list(shp), FP32,
                                          kind="ExternalOutput")

    with tile.TileContext(nc) as tc, ExitStack() as ctx, \
            nc.allow_low_precision(reason="float32r matmul operand rounding"):
        # ---- PSUM pools (8 banks total) ----
        pp_mm = ctx.enter_context(tc.tile_pool(name="pp_mm", bufs=2, space="PSUM"))
        pp_sc = ctx.enter_context(tc.tile_pool(name="pp_sc", bufs=3, space="PSUM"))
        pp_o = ctx.enter_context(tc.tile_pool(name="pp_o", bufs=2, space="PSUM"))
        pp_tr = ctx.enter_context(tc.tile_pool(name="pp_tr", bufs=1, space="PSUM"))

        sm = ctx.enter_context(tc.tile_pool(name="sm", bufs=4))
        const = ctx.enter_context(tc.tile_pool(name="const", bufs=1))

        ident = const.tile([P, P], FP32)
        make_identity(nc, ident)
        eps_t = const.tile([P, 1], FP32)
        nc.vector.memset(eps_t, EPS)

        def dump(nm, tile_):
            if dbg:
                src_ap = tile_
                if src_ap.dtype != FP32:
                    src_ap = src_ap.bitcast(FP32)
                nc.sync.dma_start(dbg_outs[nm][:], src_ap)

        def load(pool, name, chunks=1):
            t = pool.tile(list(din[name].shape), din[name].dtype,
                          name=name + "_sb", tag=name)
            if chunks == 1:
                nc.sync.dma_start(t, din[name][:])
            else:
                n1 = din[name].shape[1]
                step = n1 // chunks
                for c in range(chunks):
                    sl = slice(c * step, (c + 1) * step)
                    nc.sync.dma_start(t[:, sl], din[name][:, sl])
            return t

        def proj_T_j(dst, w_sb, x_t, b_col, j):
            ps = pp_mm.tile([P, T], FP32, name="mmps", tag="mm")
            for k in range(ND):
                nc.tensor.matmul(ps, w_sb[:, k, j * P:(j + 1) * P],
                                 x_t[:, k, :],
                                 start=(k == 0), stop=(k == ND - 1))
            nc.scalar.activation(dst[:, j, :], ps, AF.Identity,
                                 bias=b_col[:, j:j + 1])

        def proj_T(dst, w_sb, x_t, b_col):
            """dst[:, j, :] (transposed [d_out, t]) = W^T.T @ x^T + bias."""
            for j in range(ND):
                proj_T_j(dst, w_sb, x_t, b_col, j)

        def value_aug(dst, a_t, w_sb, b_bcast):
            """dst = ones-augmented V: dst[:, i, h, 0:DH] = (A @ W^T + b)."""
            for i in range(NT):
                nc.sync.dma_start(dst[:, i, :, DH:2 * DH],
                                  din["vones"][:, i])
                ps = pp_mm.tile([P, D], FP32, name="mmps", tag="mm")
                for k in range(ND):
                    nc.tensor.matmul(ps, a_t[:, k, i * P:(i + 1) * P],
                                     w_sb[:, k, :],
                                     start=(k == 0), stop=(k == ND - 1))
                nc.vector.tensor_add(
                    dst[:, i, :, 0:DH],
                    ps.rearrange("p (h e) -> p h e", h=H),
                    b_bcast.rearrange("p (h e) -> p h e", h=H))

        def score_head(h, qT, kT_or_scores, pool, mask, dbg_exp=None):
            """scores^T -> (+mask) -> exp for one head; returns exp_t tile."""
            hp, ht = (h % 2) * DH, h // 2
            exp_t = pool.tile([P, NT, T], F32R, name="expT",
                              tag="expT", bufs=3)
            for si in range(NT):
                ps = pp_sc.tile([P, T], FP32, name="scps", tag="sc")
                if callable(kT_or_scores):
                    kT_or_scores(h, si, ps)
                else:
                    kT = kT_or_scores
                    nc.tensor.matmul(
                        ps,
                        kT[hp:hp + DH, ht, si * P:(si + 1) * P],
                        qT[hp:hp + DH, ht, :],
                        start=True, stop=True)
                if mask is not None:
                    nc.vector.scalar_tensor_tensor(
                        out=exp_t[:, si, :], in0=ps, scalar=1.0,
                        in1=mask[:, si, :], op0=OP.mult, op1=OP.add)
                    nc.scalar.activation(exp_t[:, si, :], exp_t[:, si, :],
                                         AF.Exp)
                else:
                    nc.scalar.activation(exp_t[:, si, :], ps, AF.Exp)
            if h == 0 and dbg_exp:
                dump(dbg_exp, exp_t)
            return exp_t

        def finish_head(h, exp_t, vA, oT, pool):
            """(ones-augmented V) matmul -> renormalize into oT.

            V is augmented with 64 ones columns, so po[64:128] holds the
            softmax denominator replicated across partitions: renormalize is
            reciprocal + elementwise multiply, no broadcast needed.
            """
            hp, ht = (h % 2) * DH, h // 2
            po = pp_o.tile([2 * DH, T], FP32, name="ops", tag="po")
            for si in range(NT):
                nc.tensor.matmul(po, vA[:, si, h, :],
                                 exp_t[:, si, :],
                                 start=(si == 0), stop=(si == NT - 1))
            pb_sb = pool.tile([DH, T], FP32, name="pb_sb",
                              tag="pb_sb", bufs=2)
            nc.vector.reciprocal(pb_sb, po[DH:2 * DH, :])
            nc.vector.tensor_mul(oT[hp:hp + DH, ht, :], po[0:DH, :], pb_sb)

        def attention(qT, kT_or_scores, vA, oT, pool, mask=None, dbg_exp=None):
            for h in range(H):
                exp_t = score_head(h, qT, kT_or_scores, pool, mask, dbg_exp)
                finish_head(h, exp_t, vA, oT, pool)

        def out_proj_residual(oT, w_sb, resid_pb, dst):
            """dst[:, ti, :] = (resid + out-bias) + o @ W_o^T (natural)."""
            for ti in range(NT):
                ps = pp_mm.tile([P, D], FP32, name="mmps", tag="mm")
                for k in range(ND):
                    nc.tensor.matmul(ps, oT[:, k, ti * P:(ti + 1) * P],
                                     w_sb[:, k, :],
                                     start=(k == 0), stop=(k == ND - 1))
                nc.vector.scalar_tensor_tensor(
                    out=dst[:, ti, :], in0=ps, scalar=1.0,
                    in1=resid_pb[:, ti, :], op0=OP.mult, op1=OP.add)

        def layer_norm_hat(x_sb):
            """In-place: x <- (x - mean)/sqrt(var+eps), per [P, 512] tile.

            Affine (g, b) is NOT applied here: consumers either fold it into
            downstream weights (transposed path) or apply it off-path.
            """
            for ti in range(NT):
                st = sm.tile([P, 6], FP32, name="st", tag="st", bufs=4)
                nc.vector.bn_stats(st, x_sb[:, ti, :])
                mv = sm.tile([P, 2], FP32, name="mv", tag="mv", bufs=4)
                nc.vector.bn_aggr(mv, st)
                sd = sm.tile([P, 1], FP32, name="sd", tag="sd", bufs=4)
                nc.scalar.activation(sd, mv[:, 1:2], AF.Sqrt, bias=eps_t)
                nc.vector.reciprocal(sd, sd)
                nc.vector.tensor_scalar(
                    out=x_sb[:, ti, :], in0=x_sb[:, ti, :],
                    scalar1=mv[:, 0:1], scalar2=sd,
                    op0=OP.subtract, op1=OP.mult)

        def affine_into(dst, xhat, g_b, rb_b):
            """dst = xhat * g + rb on GpSimd (off the critical path)."""
            for ti in range(NT):
                nc.gpsimd.tensor_mul(dst[:, ti, :], xhat[:, ti, :], g_b)
                nc.gpsimd.tensor_add(dst[:, ti, :], dst[:, ti, :], rb_b)

        def transpose_full(dst, src):
            """dst = src^T for packed (512,512) tiles."""
            for i in range(NT):
                for j in range(ND):
                    pt = pp_tr.tile([P, P], FP32, name="trps", tag="pt")
                    nc.tensor.transpose(pt, src[:, i, j * P:(j + 1) * P], ident)
                    nc.vector.tensor_copy(dst[:, j, i * P:(i + 1) * P], pt)

        def emit_once():
          with tc.tile_pool(name="mid1", bufs=1) as mid1:
              x1h = mid1.tile([P, NT, D], FP32, name="x1h")     # LN1 x-hat
              x1t = mid1.tile([P, ND, T], F32R, name="x1t")     # x-hat^T

              # ================= self attention =================
              with tc.tile_pool(name="ph_s", bufs=1) as phs:
                  # load order = need order; first tensors chunked so the PE
                  # starts within ~2us of kernel entry
                  tgt_t = load(phs, "tgt_t", chunks=4)
                  wq = load(phs, "wq_t", chunks=4)
                  bq = load(phs, "bq")
                  wk = load(phs, "wk_t")
                  bk = load(phs, "bk")
                  wv = load(phs, "wv_t")
                  bv_b = load(phs, "bv_b")
                  mask_t = load(phs, "mask_t")
                  tgt_n = load(phs, "tgt_n")
                  wo = load(phs, "wo_t")
                  bo_b = load(phs, "bo_b")
                  # cross-attn weights prefetch in mid1 (span into phase C)
                  cwq = load(mid1, "cwq_t")
                  cwk = load(mid1, "cwk_n")
                  cwv = load(mid1, "cwv_t")
                  cwo = load(mid1, "cwo_t")
                  cbq = load(mid1, "cbq")
                  g1_b = load(mid1, "g1_b")
                  rb1_b = load(mid1, "rb1_b")

                  qT = phs.tile([P, ND, T], F32R, name="qT")
                  kT = phs.tile([P, ND, T], F32R, name="kT")
                  vA = phs.tile([P, NT, H, 2 * DH], F32R, name="vA")
                  oT = phs.tile([P, ND, T], F32R, name="oT")
                  tgtpb = phs.tile([P, NT, D], FP32, name="tgtpb")
                  for ti in range(NT):
                      nc.gpsimd.tensor_add(tgtpb[:, ti, :], tgt_n[:, ti, :],
                                           bo_b)

                  # j=0 projections first, then heads 0-1 scores: their
                  # softmax (DVE/ACT) overlaps the remaining projections.
                  proj_T_j(qT, wq, tgt_t, bq, 0)
                  proj_T_j(kT, wk, tgt_t, bk, 0)
                  exp01 = [score_head(h, qT, kT, phs, mask_t,
                                      dbg_exp="d_exp0") for h in (0, 1)]
                  value_aug(vA, tgt_t, wv, bv_b)
                  for j in range(1, ND):
                      proj_T_j(qT, wq, tgt_t, bq, j)
                      proj_T_j(kT, wk, tgt_t, bk, j)
                  for h in (0, 1):
                      finish_head(h, exp01[h], vA, oT, phs)
                  for h in range(2, H):
                      exp_t = score_head(h, qT, kT, phs, mask_t)
                      finish_head(h, exp_t, vA, oT, phs)
                  dump("d_qT", qT)
                  dump("d_kT", kT)
                  dump("d_vA", vA)
                  dump("d_oT", oT)
                  out_proj_residual(oT, wo, tgtpb, x1h)
                  layer_norm_hat(x1h)
                  transpose_full(x1t, x1h)
                  dump("d_x1", x1h)
                  dump("d_x1t", x1t)

              # ================= gated cross attention =================
              with tc.tile_pool(name="mid2", bufs=1) as mid2:
                  x2h = mid2.tile([P, NT, D], FP32, name="x2h")  # LN2 x-hat
                  x2t = mid2.tile([P, ND, T], F32R, name="x2t")
                  b1 = load(mid2, "b1")
                  g2_b = load(mid2, "g2_b")
                  rb2_b = load(mid2, "rb2_b")

                  with tc.tile_pool(name="ph_c", bufs=1) as phc:
                      mem_t = load(phc, "mem_t")
                      gate_t = load(phc, "gate_t")
                      cbv_b = load(phc, "cbv_b")

                      cqT = phc.tile([P, ND, T], F32R, name="cqT")
                      cvA = phc.tile([P, NT, H, 2 * DH], F32R, name="cvA")
                      coT = phc.tile([P, ND, T], F32R, name="coT")
                      # x1pb = true x1 + cross-out bias = x1h*g1 + rb1
                      x1pb = phc.tile([P, NT, D], FP32, name="x1pb")
                      affine_into(x1pb, x1h, g1_b, rb1_b)

                      proj_T(cqT, cwq, x1t, cbq)
                      value_aug(cvA, mem_t, cwv, cbv_b)

                      g_tiles = {}

                      def cross_scores(h, si, ps):
                          hp, ht = (h % 2) * DH, h // 2
                          if h not in g_tiles:
                              gT = phc.tile([P, ND, T], F32R, name="gT",
                                            tag="gT", bufs=2)
                              for dj in range(ND):
                                  qw = pp_mm.tile([P, T], FP32, name="mmps",
                                                  tag="mm")
                                  nc.tensor.matmul(
                                      qw,
                                      cwk[hp:hp + DH, ht, dj * P:(dj + 1) * P],
                                      cqT[hp:hp + DH, ht, :],
                                      start=True, stop=True)
                                  nc.vector.tensor_mul(gT[:, dj, :], qw,
                                                       gate_t[:, dj, :])
                              g_tiles.clear()
                              g_tiles[h] = gT
                              if h == 0:
                                  dump("d_gT0", gT)
                          gT = g_tiles[h]
                          for k in range(ND):
                              nc.tensor.matmul(
                                  ps, mem_t[:, k, si * P:(si + 1) * P],
                                  gT[:, k, :],
                                  start=(k == 0), stop=(k == ND - 1))

                      attention(cqT, cross_scores, cvA, coT, phc,
                                mask=None, dbg_exp="d_cexp0")
                      dump("d_cqT", cqT)
                      dump("d_coT", coT)
                      out_proj_residual(coT, cwo, x1pb, x2h)
                      layer_norm_hat(x2h)
                      transpose_full(x2t, x2h)
                      dump("d_x2", x2h)

                  # ================= FFN =================
                  with tc.tile_pool(name="ph_f", bufs=1) as phf:
                      w2 = load(phf, "w2_t")
                      g3_b = load(phf, "g3_b")
                      b3n_b = load(phf, "b3n_b")

                      hT = phf.tile([P, NF, T], F32R, name="hT")
                      x3 = phf.tile([P, NT, D], FP32, name="x3")
                      # x2pb = true x2 + ff2 bias = x2h*g2 + rb2
                      x2pb = phf.tile([P, NT, D], FP32, name="x2pb")
                      affine_into(x2pb, x2h, g2_b, rb2_b)

                      for fj in range(NF):
                          w1c = mid2.tile([P, ND, P], F32R, name="w1c",
                                          tag="w1c", bufs=4)
                          nc.sync.dma_start(
                              w1c, din["w1_t"][:, :, fj * P:(fj + 1) * P])
                          ps = pp_mm.tile([P, T], FP32, name="mmps", tag="mm")
                          for k in range(ND):
                              nc.tensor.matmul(ps, w1c[:, k, :],
                                               x2t[:, k, :],
                                               start=(k == 0),
                                               stop=(k == ND - 1))
                          nc.scalar.activation(hT[:, fj, :], ps, AF.Relu,
                                               bias=b1[:, fj:fj + 1])

                      dump("d_hT", hT)
                      for ti in range(NT):
                          ps = pp_mm.tile([P, D], FP32, name="mmps", tag="mm")
                          for k in range(NF):
                              nc.tensor.matmul(
                                  ps, hT[:, k, ti * P:(ti + 1) * P],
                                  w2[:, k, :],
                                  start=(k == 0), stop=(k == NF - 1))
                          nc.vector.scalar_tensor_tensor(
                              out=x3[:, ti, :], in0=ps, scalar=1.0,
                              in1=x2pb[:, ti, :], op0=OP.mult, op1=OP.add)
                      # final LN with honest affine (this is the output)
                      layer_norm_hat(x3)
                      for ti in range(NT):
                          nc.vector.tensor_mul(x3[:, ti, :], x3[:, ti, :],
                                               g3_b)
                          nc.vector.tensor_add(x3[:, ti, :], x3[:, ti, :],
                                               b3n_b)
                          nc.sync.dma_start(out_d[:, ti, :], x3[:, ti, :])

        for _ in range(iters):
            emit_once()

    return nc


# ---------------------------------------------------------------------------
# host side
# ---------------------------------------------------------------------------

def _pack(m):
    """(R, C) -> (128, R//128, C): partition-major packing."""
    m = np.ascontiguousarray(m, dtype=np.float32)
    r, c = m.shape
    return np.ascontiguousarray(m.reshape(r // P, P, c).transpose(1, 0, 2))


def _col(v):
    """(N,) -> (128, N//128) per-partition bias columns."""
    v = np.asarray(v, dtype=np.float32)
    return np.ascontiguousarray(v.reshape(-1, P).T)


def _bcast(v):
    v = np.asarray(v, dtype=np.float32)
    return np.ascontiguousarray(np.broadcast_to(v, (P, v.size)))


_CACHE = {}


def _get_nc(dbg=False, iters=1):
    key = ("nc", dbg, iters)
    if key not in _CACHE:
        nc = bacc.Bacc("TRN2", target_bir_lowering=False, debug=False,
                       enable_asserts=False, num_devices=B)
        _emit(nc, dbg=dbg, iters=iters)
        nc.compile()
        _CACHE[key] = nc
    return _CACHE[key]


def _shared_inputs(inputs):
    f32 = lambda k: np.asarray(inputs[k], np.float32)
    scale = 1.0 / np.sqrt(np.float32(DH))
    sa_w, sa_b = f32("sa_in_w"), f32("sa_in_b")
    ca_w, ca_b = f32("ca_in_w"), f32("ca_in_b")
    g1, b1n = f32("ln1_g"), f32("ln1_b")
    g2, b2n = f32("ln2_g"), f32("ln2_b")
    cwq, cbq = ca_w[0:D], ca_b[0:D]
    ff1_w, ff1_b = f32("ff1_w"), f32("ff1_b")

    # Fold LN1 affine into the cross-attn query projection:
    #   cq = (x1h*g1 + b1n) @ cwq.T + cbq  =  x1h @ (cwq*g1).T + folded-bias
    cwq_f = cwq * g1[None, :]
    cbq_f = cbq + cwq @ b1n
    # Fold LN2 affine into FFN1:
    w1_f = ff1_w * g2[None, :]
    b1_f = ff1_b + ff1_w @ b2n

    sh = {
        "mask_t": _pack(f32("tgt_mask").T),
        "gate_t": _pack(f32("gate").T),
        "wq_t": _pack(sa_w[0:D].T * scale),
        "wk_t": _pack(sa_w[D:2 * D].T),
        "wv_t": _pack(sa_w[2 * D:3 * D].T),
        "wo_t": _pack(f32("sa_out_w").T),
        "cwq_t": _pack(cwq_f.T * scale),
        "cwk_n": _pack(ca_w[D:2 * D]),
        "cwv_t": _pack(ca_w[2 * D:3 * D].T),
        "cwo_t": _pack(f32("ca_out_w").T),
        "w1_t": _pack(w1_f.T),
        "w2_t": _pack(f32("ff2_w").T),
        "bq": _col(sa_b[0:D] * scale),
        "bk": _col(sa_b[D:2 * D]),
        "cbq": _col(cbq_f * scale),
        "b1": _col(b1_f),
        "bv_b": _bcast(sa_b[2 * D:3 * D]),
        "bo_b": _bcast(f32("sa_out_b")),
        "cbv_b": _bcast(ca_b[2 * D:3 * D]),
        "g1_b": _bcast(g1),
        "rb1_b": _bcast(b1n + f32("ca_out_b")),
        "g2_b": _bcast(g2),
        "rb2_b": _bcast(b2n + f32("ff2_b")),
        "g3_b": _bcast(f32("ln3_g")),
        "b3n_b": _bcast(f32("ln3_b")),
        "vones": np.ones((P, NT, H, DH), np.float32),
    }
    return sh


def _run(inputs, trace=False, dbg=False, cores=None):
    nc = _get_nc(dbg=dbg)
    tgt = np.asarray(inputs["tgt"], np.float32)
    memory = np.asarray(inputs["memory"], np.float32)
    sh = _shared_inputs(inputs)
    core_list = list(range(B)) if cores is None else cores
    in_maps = []
    for b in core_list:
        m = dict(sh)
        m["tgt_n"] = _pack(tgt[b])
        m["tgt_t"] = _pack(tgt[b].T)
        m["mem_t"] = _pack(memory[b].T)
        in_maps.append(m)
    res = run_bass_kernel_spmd(nc, in_maps, core_list, trace=trace)
    out = np.stack([
        res.results[i]["out"].transpose(1, 0, 2).reshape(T, D)
        for i in range(len(core_list))
    ])
    return out.astype(np.float32), res


def kernel(**inputs):
    return _run(inputs, trace=False)[0]
